# revision 10
# baseline (speedup 1.0000x reference)
"""Trainium2 Bass kernel for nn_ChemGCLayer (GCN message passing layer).

Computation (matches the PyG-style reference):
    nfeats   = feats @ W1 + b1
    x        = nfeats @ Wg
    deg[d]   = #incoming edges + 1 (self loop);  dinv = deg^-1/2
    gc[d]    = dinv[d] * sum_{src in N(d)+{d}} (x[src] * dinv[src]) + bg
    combined = [nfeats | gc] @ W2 + b2
    return (combined, edges, batch)

Strategy: destination nodes are padded to 50176 = 8 * 6272 and split
across the 8 cores.  All indexed access happens on the HOST: for every
core we materialize a slot stream of message source rows
    F[slot] = (feats[src] + v) * dinv[src],   v = W1^-T b1
grouped by destination range (128 dst nodes, padded to a uniform B
blocks of 128 messages).  On device, for each 128-message block the
one-hot selector sel[m, d] = (dstlocal[m] == d) is built by a DVE
is_equal against an iota row, and the PE accumulates
    P_r = sum_b F_b^T @ sel_b          (in PSUM, per range)
which by linearity gives
    aggT_r = W1g^T @ P_r,  W1g = W1 @ Wg
    gcT_r  = aggT_r * dinv[dst]        (+ bg folded into the W2 bias)
    combined_r = nfT_own_r^T @ W2a + gcT_r^T @ W2b + ones (x) w2bias
with w2bias = bg @ W2b + b2.  No device-side gather/scatter is needed.
"""

import sys

sys.path.insert(0, "/opt/trn_rl_repo")

from contextlib import ExitStack

import ml_dtypes
import numpy as np

P = 128
N_NODES = 50000
N_CORES = 8
NPAD = 50176                  # 8 * 6272 = 392 * 128
NOWN = NPAD // N_CORES        # 6272 nodes per core
NRANGES = NOWN // P           # 49 dst ranges of 128 nodes per core
D = 128
DOUT = 256
RPC = 7                       # ranges per chunk (SBUF staging granule)
NCHUNKS = NRANGES // RPC      # 7

_PROGRAM_CACHE = {}


def _build_program(B, has_w2bias):
    import concourse.bacc as bacc
    import concourse.tile as tile
    from concourse import mybir

    bf16 = mybir.dt.bfloat16
    f32 = mybir.dt.float32
    NBC = RPC * B                      # message blocks per chunk
    NSLOT = NRANGES * B * P            # message slots per core

    nc = bacc.Bacc()
    fslot = nc.dram_tensor("fslot", [NSLOT, D], bf16, kind="ExternalInput")
    ftown = nc.dram_tensor("ftown", [P, NOWN], bf16, kind="ExternalInput")
    w1 = nc.dram_tensor("w1", [P, P], bf16, kind="ExternalInput")
    w2a = nc.dram_tensor("w2a", [P, DOUT], bf16, kind="ExternalInput")
    w1gw2b = nc.dram_tensor("w1gw2b", [P, DOUT], bf16, kind="ExternalInput")
    b1c = nc.dram_tensor("b1c", [P, 1], f32, kind="ExternalInput")
    if has_w2bias:
        w2bias = nc.dram_tensor("w2bias", [1, DOUT], f32, kind="ExternalInput")
        ones1 = nc.dram_tensor("ones1", [1, P], f32, kind="ExternalInput")
    iota = nc.dram_tensor("iota", [P, P], bf16, kind="ExternalInput")
    dinvb = nc.dram_tensor("dinvb", [P, NOWN], f32, kind="ExternalInput")
    dstloc = nc.dram_tensor("dstloc", [P, NRANGES * B], f32, kind="ExternalInput")
    out = nc.dram_tensor("combined", [NOWN, DOUT], f32, kind="ExternalOutput")

    with tile.TileContext(nc) as tc, ExitStack() as ctx:
        const = ctx.enter_context(tc.tile_pool(name="const", bufs=1))

        def _load(t, shape, dtype):
            tl = const.tile(shape, dtype, tag=t.name)
            nc.sync.dma_start(out=tl[:], in_=t[:])
            return tl

        w1_t = _load(w1, [P, P], bf16)
        w2a_t = _load(w2a, [P, DOUT], bf16)
        w1gw2b_t = _load(w1gw2b, [P, DOUT], bf16)
        b1_t = _load(b1c, [P, 1], f32)
        if has_w2bias:
            w2bias_t = _load(w2bias, [1, DOUT], f32)
            ones_t = _load(ones1, [1, P], f32)
        iota_t = _load(iota, [P, P], bf16)
        nfT_own = const.tile([P, NOWN], bf16, tag="nfT_own")

        with ExitStack() as ph:
            fpool = ph.enter_context(tc.tile_pool(name="fp", bufs=3))
            ps_nf = ph.enter_context(tc.tile_pool(name="psnf", bufs=2, space="PSUM"))
            # own nfeats: nfT = W1^T @ featsT_own (+ b1), kept resident
            for k in range(NOWN // 512):
                ft = fpool.tile([P, 512], bf16)
                nc.sync.dma_start(out=ft[:], in_=ftown[:, k * 512 : (k + 1) * 512])
                pnf = ps_nf.tile([P, 512], f32)
                nc.tensor.matmul(
                    out=pnf[:], lhsT=w1_t[:], rhs=ft[:], start=True, stop=True
                )
                nc.vector.tensor_scalar_add(
                    out=nfT_own[:, k * 512 : (k + 1) * 512], in0=pnf[:],
                    scalar1=b1_t[:, :1],
                )
            # NOWN % 512 == 128: one tail block
            k0 = (NOWN // 512) * 512
            ftl = fpool.tile([P, P], bf16, tag="ftail")
            nc.sync.dma_start(out=ftl[:], in_=ftown[:, k0:NOWN])
            pnf = ps_nf.tile([P, 512], f32)
            nc.tensor.matmul(
                out=pnf[:, :P], lhsT=w1_t[:], rhs=ftl[:], start=True, stop=True
            )
            nc.vector.tensor_scalar_add(
                out=nfT_own[:, k0:NOWN], in0=pnf[:, :P], scalar1=b1_t[:, :1]
            )

        with ExitStack() as p2:
            mpool = p2.enter_context(tc.tile_pool(name="mp", bufs=2))
            spool = p2.enter_context(tc.tile_pool(name="sp", bufs=2))
            dpool = p2.enter_context(tc.tile_pool(name="dp", bufs=2))
            gpool = p2.enter_context(tc.tile_pool(name="gp", bufs=2))
            opool = p2.enter_context(tc.tile_pool(name="op", bufs=2))
            ps_p = p2.enter_context(tc.tile_pool(name="psp", bufs=2, space="PSUM"))
            ps_c = p2.enter_context(tc.tile_pool(name="psc", bufs=2, space="PSUM"))
            for j in range(NCHUNKS):
                dl = dpool.tile([P, NBC], f32, tag="dl")
                nc.sync.dma_start(out=dl[:], in_=dstloc[:, j * NBC : (j + 1) * NBC])
                db = dpool.tile([P, RPC * P], f32, tag="db")
                nc.sync.dma_start(
                    out=db[:], in_=dinvb[:, j * RPC * P : (j + 1) * RPC * P]
                )
                # feats-slot stream on the scalar HWDGE ring, everything else
                # on the sync ring, so the two big streams run in parallel
                fs = mpool.tile([P, NBC, D], bf16)
                nc.scalar.dma_start(
                    out=fs[:, :, :],
                    in_=fslot[j * NBC * P : (j + 1) * NBC * P, :].rearrange(
                        "(b p) f -> p b f", p=P
                    ),
                )
                sels = spool.tile([P, NBC, D], bf16)
                for b in range(NBC):
                    eng = nc.vector if b % 2 == 0 else nc.gpsimd
                    eng.tensor_scalar(
                        out=sels[:, b, :], in0=iota_t[:], scalar1=dl[:, b : b + 1],
                        scalar2=None, op0=mybir.AluOpType.is_equal,
                    )
                for r in range(RPC):
                    pp = ps_p.tile([P, P], f32)
                    for bi in range(B):
                        b = r * B + bi
                        nc.tensor.matmul(
                            out=pp[:], lhsT=fs[:, b, :], rhs=sels[:, b, :],
                            start=(bi == 0), stop=(bi == B - 1),
                        )
                    # gcp[fin, d] = dinv[d] * P[fin, d]; the (gc @ W2b) term is
                    # then gcp^T @ (W1 @ Wg @ W2b), precomputed on the host
                    gcp = gpool.tile([P, P], bf16)
                    nc.vector.tensor_tensor(
                        out=gcp[:], in0=pp[:], in1=db[:, r * P : (r + 1) * P],
                        op=mybir.AluOpType.mult,
                    )
                    R = j * RPC + r
                    pc = ps_c.tile([P, DOUT], f32)
                    nc.tensor.matmul(
                        out=pc[:], lhsT=nfT_own[:, R * P : (R + 1) * P], rhs=w2a_t[:],
                        start=True, stop=False,
                    )
                    nc.tensor.matmul(
                        out=pc[:], lhsT=gcp[:], rhs=w1gw2b_t[:], start=False,
                        stop=not has_w2bias,
                    )
                    if has_w2bias:
                        nc.tensor.matmul(
                            out=pc[:], lhsT=ones_t[:], rhs=w2bias_t[:], start=False,
                            stop=True,
                        )
                    ot = opool.tile([P, DOUT], f32)
                    nc.scalar.copy(out=ot[:], in_=pc[:])
                    nc.sync.dma_start(out=out[R * P : (R + 1) * P, :], in_=ot[:])

    nc.compile()
    return nc


def _preprocess(feats, edges, W1, b1, Wg, bg, W2, b2):
    bf = ml_dtypes.bfloat16
    feats = np.asarray(feats, np.float32)
    src_all = np.concatenate([edges[0].astype(np.int64), np.arange(N_NODES)])
    dst_all = np.concatenate([edges[1].astype(np.int64), np.arange(N_NODES)])
    deg = np.bincount(dst_all, minlength=NPAD).astype(np.float64)
    dinv = np.zeros(NPAD, np.float32)
    nz = deg > 0
    dinv[nz] = (1.0 / np.sqrt(deg[nz])).astype(np.float32)

    W1a = np.asarray(W1, np.float32)
    Wga = np.asarray(Wg, np.float32)
    W2a = np.asarray(W2, np.float32)
    b1a = np.asarray(b1, np.float32)
    bga = np.asarray(bg, np.float32)
    b2a = np.asarray(b2, np.float32)
    W1gW2b = (
        W1a.astype(np.float64) @ Wga.astype(np.float64) @ W2a[D:].astype(np.float64)
    ).astype(np.float32)
    w2bias = bga @ W2a[D:] + b2a
    has_w2bias = bool(np.any(w2bias))
    if np.any(b1a):
        # v with (feats + v) @ W1g == feats @ W1g + b1 @ Wg
        v = np.linalg.solve(W1a.astype(np.float64).T, b1a.astype(np.float64))
        v = v.astype(np.float32)
    else:
        v = np.zeros(D, np.float32)

    core = dst_all // NOWN
    d_loc = dst_all - core * NOWN
    rng = d_loc >> 7
    key = core * NRANGES + rng
    order = np.argsort(key, kind="stable")
    counts = np.bincount(key, minlength=N_CORES * NRANGES)
    B = max(1, int(np.ceil(counts.max() / P)))
    NSLOT = NRANGES * B * P

    starts = np.cumsum(counts) - counts
    okey = key[order]
    within = np.arange(order.size) - starts[okey]
    slots = okey * (B * P) + within          # global slot (core-major)

    # slot -> scaled source row / local dst
    fs_all = np.zeros((N_CORES * NSLOT, D), np.float32)
    dst_flat = np.full(N_CORES * NSLOT, 200.0, np.float32)
    osrc = src_all[order]
    fs_all[slots] = (feats[osrc] + v) * dinv[osrc][:, None]
    dst_flat[slots] = (d_loc[order] & 127).astype(np.float32)
    fs_all = fs_all.astype(bf)

    featsT_pad = np.zeros((NPAD, D), np.float32)
    featsT_pad[:N_NODES] = feats
    fT = np.ascontiguousarray(featsT_pad.T)           # [128, NPAD] f32

    common = dict(
        w1=W1a.astype(bf),
        w2a=W2a[:D].astype(bf),
        w1gw2b=W1gW2b.astype(bf),
        b1c=b1a[:, None].astype(np.float32),
        iota=np.tile(np.arange(P), (P, 1)).astype(bf),
    )
    if has_w2bias:
        common["w2bias"] = w2bias[None, :].astype(np.float32)
        common["ones1"] = np.ones((1, P), np.float32)
    in_maps = []
    for c in range(N_CORES):
        dinv_own = dinv[c * NOWN : (c + 1) * NOWN]
        in_maps.append(
            dict(
                common,
                fslot=fs_all[c * NSLOT : (c + 1) * NSLOT],
                ftown=np.ascontiguousarray(fT[:, c * NOWN : (c + 1) * NOWN]).astype(bf),
                dinvb=np.broadcast_to(dinv_own, (P, NOWN)).copy(),
                dstloc=np.ascontiguousarray(
                    dst_flat[c * NSLOT : (c + 1) * NSLOT].reshape(NRANGES * B, P).T
                ),
            )
        )
    return in_maps, B, has_w2bias


def kernel(feats, edges, batch, W1, b1, Wg, bg, W2, b2):
    from concourse.bass_utils import run_bass_kernel_spmd

    feats = np.asarray(feats)
    edges_np = np.asarray(edges)
    in_maps, B, has_w2bias = _preprocess(feats, edges_np, W1, b1, Wg, bg, W2, b2)
    key = (B, has_w2bias)
    nc = _PROGRAM_CACHE.get(key)
    if nc is None:
        nc = _build_program(B, has_w2bias)
        _PROGRAM_CACHE[key] = nc
    res = run_bass_kernel_spmd(nc, in_maps, core_ids=list(range(N_CORES)))
    combined = np.concatenate(
        [res.results[c]["combined"] for c in range(N_CORES)], axis=0
    )[:N_NODES]
    return combined.astype(np.float32), edges, batch


# revision 11
# speedup vs baseline: 4.3581x; 4.3581x over previous
"""Trainium2 Bass kernel for nn_ChemGCLayer (GCN message passing layer).

Computation (matches the PyG-style reference):
    nfeats   = feats @ W1 + b1
    x        = nfeats @ Wg
    deg[d]   = #incoming edges + 1 (self loop);  dinv = deg^-1/2
    gc[d]    = dinv[d] * sum_{src in N(d)+{d}} (x[src] * dinv[src]) + bg
    combined = [nfeats | gc] @ W2 + b2
    return (combined, edges, batch)

Strategy: destination nodes are padded to 50176 = 8 * 6272 and split
across the 8 cores.  All indexed access happens on the HOST: for every
core we materialize a slot stream of message source rows
    F[slot] = (feats[src] + v) * dinv[src],   v = W1^-T b1
grouped by destination range (128 dst nodes, padded to a uniform B
blocks of 128 messages).  On device, for each 128-message block the
one-hot selector sel[m, d] = (dstlocal[m] == d) is built by a DVE
is_equal against an iota row, and the PE accumulates
    P_r = sum_b F_b^T @ sel_b          (in PSUM, per range)
which by linearity gives
    aggT_r = W1g^T @ P_r,  W1g = W1 @ Wg
    gcT_r  = aggT_r * dinv[dst]        (+ bg folded into the W2 bias)
    combined_r = nfT_own_r^T @ W2a + gcT_r^T @ W2b + ones (x) w2bias
with w2bias = bg @ W2b + b2.  No device-side gather/scatter is needed.
"""

import sys

sys.path.insert(0, "/opt/trn_rl_repo")

from contextlib import ExitStack

import ml_dtypes
import numpy as np

P = 128
N_NODES = 50000
N_CORES = 8
NPAD = 50176                  # 8 * 6272 = 392 * 128
NOWN = NPAD // N_CORES        # 6272 nodes per core
NRANGES = NOWN // P           # 49 dst ranges of 128 nodes per core
D = 128
DOUT = 256
RPC = 7                       # ranges per chunk (SBUF staging granule)
NCHUNKS = NRANGES // RPC      # 7

_PROGRAM_CACHE = {}


def _build_program(B, has_w2bias):
    import concourse.bacc as bacc
    import concourse.tile as tile
    from concourse import mybir

    bf16 = mybir.dt.bfloat16
    f32 = mybir.dt.float32
    NBC = RPC * B                      # message blocks per chunk
    NSLOT = NRANGES * B * P            # message slots per core

    nc = bacc.Bacc()
    fslot = nc.dram_tensor("fslot", [NSLOT, D], bf16, kind="ExternalInput")
    ftown = nc.dram_tensor("ftown", [P, NOWN], bf16, kind="ExternalInput")
    w1 = nc.dram_tensor("w1", [P, P], bf16, kind="ExternalInput")
    w2a = nc.dram_tensor("w2a", [P, DOUT], bf16, kind="ExternalInput")
    w1gw2b = nc.dram_tensor("w1gw2b", [P, DOUT], bf16, kind="ExternalInput")
    b1c = nc.dram_tensor("b1c", [P, 1], f32, kind="ExternalInput")
    if has_w2bias:
        w2bias = nc.dram_tensor("w2bias", [1, DOUT], f32, kind="ExternalInput")
        ones1 = nc.dram_tensor("ones1", [1, P], f32, kind="ExternalInput")
    iota = nc.dram_tensor("iota", [P, P], bf16, kind="ExternalInput")
    dinvb = nc.dram_tensor("dinvb", [P, NOWN], f32, kind="ExternalInput")
    dstloc = nc.dram_tensor("dstloc", [P, NRANGES * B], f32, kind="ExternalInput")
    out = nc.dram_tensor("combined", [NOWN, DOUT], f32, kind="ExternalOutput")

    with tile.TileContext(nc) as tc, ExitStack() as ctx:
        const = ctx.enter_context(tc.tile_pool(name="const", bufs=1))

        def _load(t, shape, dtype):
            tl = const.tile(shape, dtype, tag=t.name)
            nc.sync.dma_start(out=tl[:], in_=t[:])
            return tl

        w1_t = _load(w1, [P, P], bf16)
        w2a_t = _load(w2a, [P, DOUT], bf16)
        w1gw2b_t = _load(w1gw2b, [P, DOUT], bf16)
        b1_t = _load(b1c, [P, 1], f32)
        if has_w2bias:
            w2bias_t = _load(w2bias, [1, DOUT], f32)
            ones_t = _load(ones1, [1, P], f32)
        iota_t = _load(iota, [P, P], bf16)
        nfT_own = const.tile([P, NOWN], bf16, tag="nfT_own")

        with ExitStack() as ph:
            fpool = ph.enter_context(tc.tile_pool(name="fp", bufs=3))
            ps_nf = ph.enter_context(tc.tile_pool(name="psnf", bufs=2, space="PSUM"))
            # own nfeats: nfT = W1^T @ featsT_own (+ b1), kept resident
            for k in range(NOWN // 512):
                ft = fpool.tile([P, 512], bf16)
                nc.sync.dma_start(out=ft[:], in_=ftown[:, k * 512 : (k + 1) * 512])
                pnf = ps_nf.tile([P, 512], f32)
                nc.tensor.matmul(
                    out=pnf[:], lhsT=w1_t[:], rhs=ft[:], start=True, stop=True
                )
                nc.vector.tensor_scalar_add(
                    out=nfT_own[:, k * 512 : (k + 1) * 512], in0=pnf[:],
                    scalar1=b1_t[:, :1],
                )
            # NOWN % 512 == 128: one tail block
            k0 = (NOWN // 512) * 512
            ftl = fpool.tile([P, P], bf16, tag="ftail")
            nc.sync.dma_start(out=ftl[:], in_=ftown[:, k0:NOWN])
            pnf = ps_nf.tile([P, 512], f32)
            nc.tensor.matmul(
                out=pnf[:, :P], lhsT=w1_t[:], rhs=ftl[:], start=True, stop=True
            )
            nc.vector.tensor_scalar_add(
                out=nfT_own[:, k0:NOWN], in0=pnf[:, :P], scalar1=b1_t[:, :1]
            )

        with ExitStack() as p2:
            mpool = p2.enter_context(tc.tile_pool(name="mp", bufs=2))
            spool = p2.enter_context(tc.tile_pool(name="sp", bufs=2))
            dpool = p2.enter_context(tc.tile_pool(name="dp", bufs=2))
            gpool = p2.enter_context(tc.tile_pool(name="gp", bufs=2))
            opool = p2.enter_context(tc.tile_pool(name="op", bufs=2))
            ps_p = p2.enter_context(tc.tile_pool(name="psp", bufs=2, space="PSUM"))
            ps_c = p2.enter_context(tc.tile_pool(name="psc", bufs=2, space="PSUM"))
            for j in range(NCHUNKS):
                dl = dpool.tile([P, NBC], f32, tag="dl")
                nc.sync.dma_start(out=dl[:], in_=dstloc[:, j * NBC : (j + 1) * NBC])
                db = dpool.tile([P, RPC * P], f32, tag="db")
                nc.sync.dma_start(
                    out=db[:], in_=dinvb[:, j * RPC * P : (j + 1) * RPC * P]
                )
                # feats-slot stream on the scalar HWDGE ring, everything else
                # on the sync ring, so the two big streams run in parallel
                fs = mpool.tile([P, NBC, D], bf16)
                nc.scalar.dma_start(
                    out=fs[:, :, :],
                    in_=fslot[j * NBC * P : (j + 1) * NBC * P, :].rearrange(
                        "(b p) f -> p b f", p=P
                    ),
                )
                sels = spool.tile([P, NBC, D], bf16)
                for b in range(NBC):
                    nc.vector.tensor_scalar(
                        out=sels[:, b, :], in0=iota_t[:], scalar1=dl[:, b : b + 1],
                        scalar2=None, op0=mybir.AluOpType.is_equal,
                    )
                for r in range(RPC):
                    pp = ps_p.tile([P, P], f32)
                    for bi in range(B):
                        b = r * B + bi
                        nc.tensor.matmul(
                            out=pp[:], lhsT=fs[:, b, :], rhs=sels[:, b, :],
                            start=(bi == 0), stop=(bi == B - 1),
                        )
                    # gcp[fin, d] = dinv[d] * P[fin, d]; the (gc @ W2b) term is
                    # then gcp^T @ (W1 @ Wg @ W2b), precomputed on the host
                    gcp = gpool.tile([P, P], bf16)
                    nc.vector.tensor_tensor(
                        out=gcp[:], in0=pp[:], in1=db[:, r * P : (r + 1) * P],
                        op=mybir.AluOpType.mult,
                    )
                    R = j * RPC + r
                    pc = ps_c.tile([P, DOUT], f32)
                    nc.tensor.matmul(
                        out=pc[:], lhsT=nfT_own[:, R * P : (R + 1) * P], rhs=w2a_t[:],
                        start=True, stop=False,
                    )
                    nc.tensor.matmul(
                        out=pc[:], lhsT=gcp[:], rhs=w1gw2b_t[:], start=False,
                        stop=not has_w2bias,
                    )
                    if has_w2bias:
                        nc.tensor.matmul(
                            out=pc[:], lhsT=ones_t[:], rhs=w2bias_t[:], start=False,
                            stop=True,
                        )
                    ot = opool.tile([P, DOUT], f32)
                    nc.scalar.copy(out=ot[:], in_=pc[:])
                    nc.sync.dma_start(out=out[R * P : (R + 1) * P, :], in_=ot[:])

    nc.compile()
    return nc


def _preprocess(feats, edges, W1, b1, Wg, bg, W2, b2):
    bf = ml_dtypes.bfloat16
    feats = np.asarray(feats, np.float32)
    src_all = np.concatenate([edges[0].astype(np.int64), np.arange(N_NODES)])
    dst_all = np.concatenate([edges[1].astype(np.int64), np.arange(N_NODES)])
    deg = np.bincount(dst_all, minlength=NPAD).astype(np.float64)
    dinv = np.zeros(NPAD, np.float32)
    nz = deg > 0
    dinv[nz] = (1.0 / np.sqrt(deg[nz])).astype(np.float32)

    W1a = np.asarray(W1, np.float32)
    Wga = np.asarray(Wg, np.float32)
    W2a = np.asarray(W2, np.float32)
    b1a = np.asarray(b1, np.float32)
    bga = np.asarray(bg, np.float32)
    b2a = np.asarray(b2, np.float32)
    W1gW2b = (
        W1a.astype(np.float64) @ Wga.astype(np.float64) @ W2a[D:].astype(np.float64)
    ).astype(np.float32)
    w2bias = bga @ W2a[D:] + b2a
    has_w2bias = bool(np.any(w2bias))
    if np.any(b1a):
        # v with (feats + v) @ W1g == feats @ W1g + b1 @ Wg
        v = np.linalg.solve(W1a.astype(np.float64).T, b1a.astype(np.float64))
        v = v.astype(np.float32)
    else:
        v = np.zeros(D, np.float32)

    core = dst_all // NOWN
    d_loc = dst_all - core * NOWN
    rng = d_loc >> 7
    key = core * NRANGES + rng
    order = np.argsort(key, kind="stable")
    counts = np.bincount(key, minlength=N_CORES * NRANGES)
    B = max(1, int(np.ceil(counts.max() / P)))
    NSLOT = NRANGES * B * P

    starts = np.cumsum(counts) - counts
    okey = key[order]
    within = np.arange(order.size) - starts[okey]
    slots = okey * (B * P) + within          # global slot (core-major)

    # slot -> scaled source row / local dst
    fs_all = np.zeros((N_CORES * NSLOT, D), np.float32)
    dst_flat = np.full(N_CORES * NSLOT, 200.0, np.float32)
    osrc = src_all[order]
    fs_all[slots] = (feats[osrc] + v) * dinv[osrc][:, None]
    dst_flat[slots] = (d_loc[order] & 127).astype(np.float32)
    fs_all = fs_all.astype(bf)

    featsT_pad = np.zeros((NPAD, D), np.float32)
    featsT_pad[:N_NODES] = feats
    fT = np.ascontiguousarray(featsT_pad.T)           # [128, NPAD] f32

    common = dict(
        w1=W1a.astype(bf),
        w2a=W2a[:D].astype(bf),
        w1gw2b=W1gW2b.astype(bf),
        b1c=b1a[:, None].astype(np.float32),
        iota=np.tile(np.arange(P), (P, 1)).astype(bf),
    )
    if has_w2bias:
        common["w2bias"] = w2bias[None, :].astype(np.float32)
        common["ones1"] = np.ones((1, P), np.float32)
    in_maps = []
    for c in range(N_CORES):
        dinv_own = dinv[c * NOWN : (c + 1) * NOWN]
        in_maps.append(
            dict(
                common,
                fslot=fs_all[c * NSLOT : (c + 1) * NSLOT],
                ftown=np.ascontiguousarray(fT[:, c * NOWN : (c + 1) * NOWN]).astype(bf),
                dinvb=np.broadcast_to(dinv_own, (P, NOWN)).copy(),
                dstloc=np.ascontiguousarray(
                    dst_flat[c * NSLOT : (c + 1) * NSLOT].reshape(NRANGES * B, P).T
                ),
            )
        )
    return in_maps, B, has_w2bias


def kernel(feats, edges, batch, W1, b1, Wg, bg, W2, b2):
    from concourse.bass_utils import run_bass_kernel_spmd

    feats = np.asarray(feats)
    edges_np = np.asarray(edges)
    in_maps, B, has_w2bias = _preprocess(feats, edges_np, W1, b1, Wg, bg, W2, b2)
    key = (B, has_w2bias)
    nc = _PROGRAM_CACHE.get(key)
    if nc is None:
        nc = _build_program(B, has_w2bias)
        _PROGRAM_CACHE[key] = nc
    res = run_bass_kernel_spmd(nc, in_maps, core_ids=list(range(N_CORES)))
    combined = np.concatenate(
        [res.results[c]["combined"] for c in range(N_CORES)], axis=0
    )[:N_NODES]
    return combined.astype(np.float32), edges, batch


# revision 12
# speedup vs baseline: 4.3772x; 1.0044x over previous
"""Trainium2 Bass kernel for nn_ChemGCLayer (GCN message passing layer).

Computation (matches the PyG-style reference):
    nfeats   = feats @ W1 + b1
    x        = nfeats @ Wg
    deg[d]   = #incoming edges + 1 (self loop);  dinv = deg^-1/2
    gc[d]    = dinv[d] * sum_{src in N(d)+{d}} (x[src] * dinv[src]) + bg
    combined = [nfeats | gc] @ W2 + b2
    return (combined, edges, batch)

Strategy: destination nodes are padded to 50176 = 8 * 6272 and split
across the 8 cores.  All indexed access happens on the HOST: for every
core we materialize a slot stream of message source rows
    F[slot] = (feats[src] + v) * dinv[src],   v = W1^-T b1
grouped by destination range (128 dst nodes, padded to a uniform B
blocks of 128 messages).  On device, for each 128-message block the
one-hot selector sel[m, d] = (dstlocal[m] == d) is built by a DVE
is_equal against an iota row, and the PE accumulates
    P_r = sum_b F_b^T @ sel_b          (in PSUM, per range)
which by linearity gives
    aggT_r = W1g^T @ P_r,  W1g = W1 @ Wg
    gcT_r  = aggT_r * dinv[dst]        (+ bg folded into the W2 bias)
    combined_r = nfT_own_r^T @ W2a + gcT_r^T @ W2b + ones (x) w2bias
with w2bias = bg @ W2b + b2.  No device-side gather/scatter is needed.
"""

import sys

sys.path.insert(0, "/opt/trn_rl_repo")

from contextlib import ExitStack

import ml_dtypes
import numpy as np

P = 128
N_NODES = 50000
N_CORES = 8
NPAD = 50176                  # 8 * 6272 = 392 * 128
NOWN = NPAD // N_CORES        # 6272 nodes per core
NRANGES = NOWN // P           # 49 dst ranges of 128 nodes per core
D = 128
DOUT = 256
RPC = 7                       # ranges per chunk (SBUF staging granule)
NCHUNKS = NRANGES // RPC      # 7

_PROGRAM_CACHE = {}


def _build_program(B, has_w2bias):
    import concourse.bacc as bacc
    import concourse.tile as tile
    from concourse import mybir

    bf16 = mybir.dt.bfloat16
    f32 = mybir.dt.float32
    NBC = RPC * B                      # message blocks per chunk
    NSLOT = NRANGES * B * P            # message slots per core

    nc = bacc.Bacc()
    fslot = nc.dram_tensor("fslot", [NSLOT, D], bf16, kind="ExternalInput")
    ftown = nc.dram_tensor("ftown", [P, NOWN], bf16, kind="ExternalInput")
    w1 = nc.dram_tensor("w1", [P, P], bf16, kind="ExternalInput")
    w2a = nc.dram_tensor("w2a", [P, DOUT], bf16, kind="ExternalInput")
    w1gw2b = nc.dram_tensor("w1gw2b", [P, DOUT], bf16, kind="ExternalInput")
    b1c = nc.dram_tensor("b1c", [P, 1], f32, kind="ExternalInput")
    if has_w2bias:
        w2bias = nc.dram_tensor("w2bias", [1, DOUT], f32, kind="ExternalInput")
        ones1 = nc.dram_tensor("ones1", [1, P], f32, kind="ExternalInput")
    iota = nc.dram_tensor("iota", [P, P], bf16, kind="ExternalInput")
    dinvb = nc.dram_tensor("dinvb", [P, NOWN], f32, kind="ExternalInput")
    dstloc = nc.dram_tensor("dstloc", [P, NRANGES * B], f32, kind="ExternalInput")
    out = nc.dram_tensor("combined", [NOWN, DOUT], f32, kind="ExternalOutput")

    with tile.TileContext(nc) as tc, ExitStack() as ctx:
        const = ctx.enter_context(tc.tile_pool(name="const", bufs=1))

        def _load(t, shape, dtype):
            tl = const.tile(shape, dtype, tag=t.name)
            nc.sync.dma_start(out=tl[:], in_=t[:])
            return tl

        w1_t = _load(w1, [P, P], bf16)
        w2a_t = _load(w2a, [P, DOUT], bf16)
        w1gw2b_t = _load(w1gw2b, [P, DOUT], bf16)
        b1_t = _load(b1c, [P, 1], f32)
        if has_w2bias:
            w2bias_t = _load(w2bias, [1, DOUT], f32)
            ones_t = _load(ones1, [1, P], f32)
        iota_t = _load(iota, [P, P], bf16)
        nfT_own = const.tile([P, NOWN], bf16, tag="nfT_own")

        with ExitStack() as ph:
            fpool = ph.enter_context(tc.tile_pool(name="fp", bufs=3))
            ps_nf = ph.enter_context(tc.tile_pool(name="psnf", bufs=2, space="PSUM"))
            # own nfeats: nfT = W1^T @ featsT_own (+ b1), kept resident
            for k in range(NOWN // 512):
                ft = fpool.tile([P, 512], bf16)
                nc.sync.dma_start(out=ft[:], in_=ftown[:, k * 512 : (k + 1) * 512])
                pnf = ps_nf.tile([P, 512], f32)
                nc.tensor.matmul(
                    out=pnf[:], lhsT=w1_t[:], rhs=ft[:], start=True, stop=True
                )
                nc.vector.tensor_scalar_add(
                    out=nfT_own[:, k * 512 : (k + 1) * 512], in0=pnf[:],
                    scalar1=b1_t[:, :1],
                )
            # NOWN % 512 == 128: one tail block
            k0 = (NOWN // 512) * 512
            ftl = fpool.tile([P, P], bf16, tag="ftail")
            nc.sync.dma_start(out=ftl[:], in_=ftown[:, k0:NOWN])
            pnf = ps_nf.tile([P, 512], f32)
            nc.tensor.matmul(
                out=pnf[:, :P], lhsT=w1_t[:], rhs=ftl[:], start=True, stop=True
            )
            nc.vector.tensor_scalar_add(
                out=nfT_own[:, k0:NOWN], in0=pnf[:, :P], scalar1=b1_t[:, :1]
            )

        with ExitStack() as p2:
            mpool = p2.enter_context(tc.tile_pool(name="mp", bufs=4))
            spool = p2.enter_context(tc.tile_pool(name="sp", bufs=4))
            gpool = p2.enter_context(tc.tile_pool(name="gp", bufs=2))
            opool = p2.enter_context(tc.tile_pool(name="op", bufs=2))
            ps_p = p2.enter_context(tc.tile_pool(name="psp", bufs=2, space="PSUM"))
            ps_c = p2.enter_context(tc.tile_pool(name="psc", bufs=2, space="PSUM"))
            # destination-local ids and dinv broadcasts stay resident
            dl = const.tile([P, NRANGES * B], f32, tag="dl")
            nc.sync.dma_start(out=dl[:], in_=dstloc[:, :])
            db = const.tile([P, NOWN], f32, tag="db")
            nc.sync.dma_start(out=db[:], in_=dinvb[:, :])
            for R in range(NRANGES):
                # feats-slot stream on the scalar HWDGE ring so it runs in
                # parallel with the sync-ring output writes
                fs = mpool.tile([P, B, D], bf16)
                nc.scalar.dma_start(
                    out=fs[:, :, :],
                    in_=fslot[R * B * P : (R + 1) * B * P, :].rearrange(
                        "(b p) f -> p b f", p=P
                    ),
                )
                sels = spool.tile([P, B, D], bf16)
                for bi in range(B):
                    nc.vector.tensor_scalar(
                        out=sels[:, bi, :], in0=iota_t[:],
                        scalar1=dl[:, R * B + bi : R * B + bi + 1],
                        scalar2=None, op0=mybir.AluOpType.is_equal,
                    )
                pp = ps_p.tile([P, P], f32)
                for bi in range(B):
                    nc.tensor.matmul(
                        out=pp[:], lhsT=fs[:, bi, :], rhs=sels[:, bi, :],
                        start=(bi == 0), stop=(bi == B - 1),
                    )
                # gcp[fin, d] = dinv[d] * P[fin, d]; the (gc @ W2b) term is
                # then gcp^T @ (W1 @ Wg @ W2b), precomputed on the host
                gcp = gpool.tile([P, P], bf16)
                nc.vector.tensor_tensor(
                    out=gcp[:], in0=pp[:], in1=db[:, R * P : (R + 1) * P],
                    op=mybir.AluOpType.mult,
                )
                pc = ps_c.tile([P, DOUT], f32)
                nc.tensor.matmul(
                    out=pc[:], lhsT=nfT_own[:, R * P : (R + 1) * P], rhs=w2a_t[:],
                    start=True, stop=False,
                )
                nc.tensor.matmul(
                    out=pc[:], lhsT=gcp[:], rhs=w1gw2b_t[:], start=False,
                    stop=not has_w2bias,
                )
                if has_w2bias:
                    nc.tensor.matmul(
                        out=pc[:], lhsT=ones_t[:], rhs=w2bias_t[:], start=False,
                        stop=True,
                    )
                ot = opool.tile([P, DOUT], f32)
                nc.scalar.copy(out=ot[:], in_=pc[:])
                nc.sync.dma_start(out=out[R * P : (R + 1) * P, :], in_=ot[:])

    nc.compile()
    return nc


def _preprocess(feats, edges, W1, b1, Wg, bg, W2, b2):
    bf = ml_dtypes.bfloat16
    feats = np.asarray(feats, np.float32)
    src_all = np.concatenate([edges[0].astype(np.int64), np.arange(N_NODES)])
    dst_all = np.concatenate([edges[1].astype(np.int64), np.arange(N_NODES)])
    deg = np.bincount(dst_all, minlength=NPAD).astype(np.float64)
    dinv = np.zeros(NPAD, np.float32)
    nz = deg > 0
    dinv[nz] = (1.0 / np.sqrt(deg[nz])).astype(np.float32)

    W1a = np.asarray(W1, np.float32)
    Wga = np.asarray(Wg, np.float32)
    W2a = np.asarray(W2, np.float32)
    b1a = np.asarray(b1, np.float32)
    bga = np.asarray(bg, np.float32)
    b2a = np.asarray(b2, np.float32)
    W1gW2b = (
        W1a.astype(np.float64) @ Wga.astype(np.float64) @ W2a[D:].astype(np.float64)
    ).astype(np.float32)
    w2bias = bga @ W2a[D:] + b2a
    has_w2bias = bool(np.any(w2bias))
    if np.any(b1a):
        # v with (feats + v) @ W1g == feats @ W1g + b1 @ Wg
        v = np.linalg.solve(W1a.astype(np.float64).T, b1a.astype(np.float64))
        v = v.astype(np.float32)
    else:
        v = np.zeros(D, np.float32)

    core = dst_all // NOWN
    d_loc = dst_all - core * NOWN
    rng = d_loc >> 7
    key = core * NRANGES + rng
    order = np.argsort(key, kind="stable")
    counts = np.bincount(key, minlength=N_CORES * NRANGES)
    B = max(1, int(np.ceil(counts.max() / P)))
    NSLOT = NRANGES * B * P

    starts = np.cumsum(counts) - counts
    okey = key[order]
    within = np.arange(order.size) - starts[okey]
    slots = okey * (B * P) + within          # global slot (core-major)

    # slot -> scaled source row / local dst
    fs_all = np.zeros((N_CORES * NSLOT, D), np.float32)
    dst_flat = np.full(N_CORES * NSLOT, 200.0, np.float32)
    osrc = src_all[order]
    fs_all[slots] = (feats[osrc] + v) * dinv[osrc][:, None]
    dst_flat[slots] = (d_loc[order] & 127).astype(np.float32)
    fs_all = fs_all.astype(bf)

    featsT_pad = np.zeros((NPAD, D), np.float32)
    featsT_pad[:N_NODES] = feats
    fT = np.ascontiguousarray(featsT_pad.T)           # [128, NPAD] f32

    common = dict(
        w1=W1a.astype(bf),
        w2a=W2a[:D].astype(bf),
        w1gw2b=W1gW2b.astype(bf),
        b1c=b1a[:, None].astype(np.float32),
        iota=np.tile(np.arange(P), (P, 1)).astype(bf),
    )
    if has_w2bias:
        common["w2bias"] = w2bias[None, :].astype(np.float32)
        common["ones1"] = np.ones((1, P), np.float32)
    in_maps = []
    for c in range(N_CORES):
        dinv_own = dinv[c * NOWN : (c + 1) * NOWN]
        in_maps.append(
            dict(
                common,
                fslot=fs_all[c * NSLOT : (c + 1) * NSLOT],
                ftown=np.ascontiguousarray(fT[:, c * NOWN : (c + 1) * NOWN]).astype(bf),
                dinvb=np.broadcast_to(dinv_own, (P, NOWN)).copy(),
                dstloc=np.ascontiguousarray(
                    dst_flat[c * NSLOT : (c + 1) * NSLOT].reshape(NRANGES * B, P).T
                ),
            )
        )
    return in_maps, B, has_w2bias


def kernel(feats, edges, batch, W1, b1, Wg, bg, W2, b2):
    from concourse.bass_utils import run_bass_kernel_spmd

    feats = np.asarray(feats)
    edges_np = np.asarray(edges)
    in_maps, B, has_w2bias = _preprocess(feats, edges_np, W1, b1, Wg, bg, W2, b2)
    key = (B, has_w2bias)
    nc = _PROGRAM_CACHE.get(key)
    if nc is None:
        nc = _build_program(B, has_w2bias)
        _PROGRAM_CACHE[key] = nc
    res = run_bass_kernel_spmd(nc, in_maps, core_ids=list(range(N_CORES)))
    combined = np.concatenate(
        [res.results[c]["combined"] for c in range(N_CORES)], axis=0
    )[:N_NODES]
    return combined.astype(np.float32), edges, batch
